# revision 35
# baseline (speedup 1.0000x reference)
"""StyleGAN-style modulated 3x3 conv on 8 Trainium2 NeuronCores.

Problem: y = conv2d(x, kernel * (style+1) / demod), SAME padding,
  x [B=8, H=128, W=128, C=256], kernel [3,3,C=256,F=256],
  style [B,1,1,C], demod[f] = sqrt(sum_{ky,kx,c} wmod^2 + 1e-8).

Sharding: data-parallel over batch B - each of the 8 cores convolves one
sample with its own modulated/demodulated kernel. No cross-core comm.

Algorithm: 1-D Winograd F(2,3) along H. The 3 ky taps collapse into 4
transformed planes, so each output row-pair needs 4x3(kx)x2(ch) = 24
matmul passes instead of the direct conv's 36: PE streamed columns drop
1.5x (599k -> 399k per core, ~166us at 2.4 GHz).

  forward (DVE, bf16 2x): per tile-row i (output rows 2i, 2i+1), with
    d_a = xp[2i+a] (xp = zero-padded [C,130,130] bf16 image):
      V0 = d0-d2, V1 = d1+d2, V2 = d2-d1, V3 = d1-d3
    produced per-chunk for the first two chunks (fast availability at
    the head), then two chunks per op.
  matmuls (PE): M_k[f,p] += sum_{kx,c} wt[k,kx,c,f] * V_k[c,p+kx-1],
    kx shift applied on the PSUM *output* AP (out cols 2-kx..) so the
    moving operand is one contiguous [128,390] AP per (k,ch) and tile
    seams land on junk pad columns. Banks filled in order k=0,3,1,2 to
    match weight-combo readiness at the head.
  weights (DVE bf16, head, k-ordered): wt0 = g0*s, wt3 = g2*s,
    t = g0+g2, t2 = t+g1, t3 = t-g1, wt1 = t2*s/2, wt2 = t3*s/2.
  demod: sq = ACT.Square(wraw, scale=s) one op per c-half (= wmod^2,
    bf16); d2[f, fh] accumulated into one [128,2] PSUM tile via 36 tiny
    matmuls against a ones column, emitted in 4 parts interleaved with
    the first group's banks so invd is ready before the first drain;
    invd = 1/sqrt(d2+1e-8).
  inverse (ACT+DVE+GPSIMD, fused into PSUM drain, demod applied as
  per-partition scale):
    y_even = (M0+M1+M2)*invd, y_odd = (M1-M2-M3)*invd via
      s1 = ACT.copy(M1, scale=iv), s2 = ACT.copy(M2, scale=iv)
      a = DVE.STT(M0*iv + s1);  b = DVE.STT(M3*(-iv) + s1)
      y_e = GPSIMD.add(a, s2);  y_o = GPSIMD.sub(b, s2)
    y_e/y_o written to halves of one tile -> single DMA per group,
    alternating gpsimd/sync trigger queues.

Host does layout-only marshalling: shard over B, transpose+zero-pad x
to [ch,128,130,130] bf16, reorder kernel taps kx-major (bf16); gather
interleaves even/odd output row planes and strips pad columns.

Measured: ~197.8-198.7 us HW exec (8 cores, traced), rel err 5.4e-3, vs
the direct-conv baseline's 285.6 us / 2.3e-3 (same measurement): 1.44x.
Breakdown: ~7 us fixed NEFF preamble, ~9 us head (DMA latency + V/
weight prep, PE HAM-prewarmed by dummy matmuls), ~177 us PE-bound MM
region (>99% tensor-engine occupancy, streaming floor 166 us), ~14 us
drain tail + NEFF epilogue. Occasional ~20% slower runs are the chip's
P0 power-state downclock (2.4 -> 2.0 GHz), not kernel-dependent.
"""

import sys
import os

for _p in ("/opt/trn_rl_repo", "/root/.axon_site", "/root/.axon_site/_ro/trn_rl_repo",
           "/root/.axon_site/_ro/pypackages"):
    if os.path.isdir(_p) and _p not in sys.path:
        sys.path.append(_p)

import numpy as np

B, H, W, C, F = 8, 128, 128, 256, 256
CH = C // 128                  # c-half count (contraction tiled by 128)
ROWS, COLS = H + 2, W + 2      # padded image dims
TR = H // 2                    # 64 Winograd tile-rows (2 output rows each)
CTR = 3                        # tile-rows per PSUM group (3*130=390 <= 512)
NCHUNK = (TR + CTR - 1) // CTR # 22 (21 full + 1 single-row)
VG = 2 * CTR                   # tile-rows per steady-state V op (2 chunks)
N_CORES = 8

_COMPILED = {}

V_DEFS = [(0, 2, "sub"), (1, 2, "add"), (2, 1, "sub"), (1, 3, "sub")]


def _chunk_rows(c0):
    r0 = CTR * c0
    r1 = min(TR, r0 + CTR)
    return r0, r1


def _build_nc():
    import concourse.bacc as bacc
    import concourse.mybir as mybir
    import concourse.tile as tile

    f32 = mybir.dt.float32
    bf16 = mybir.dt.bfloat16
    AF = mybir.ActivationFunctionType
    ALU = mybir.AluOpType

    nc = bacc.Bacc("TRN2", target_bir_lowering=False, debug=False,
                   num_devices=N_CORES)

    xt_d = nc.dram_tensor("xt", [CH, 128, ROWS * COLS], bf16,
                          kind="ExternalInput").ap()
    st_d = nc.dram_tensor("st", [128, CH], f32, kind="ExternalInput").ap()
    # raw weights bf16, tap axis kx-major (t = 3*kx + ky)
    wk_d = nc.dram_tensor("wk", [CH, 128, 9, F], bf16,
                          kind="ExternalInput").ap()
    # merged even/odd output row planes [f_half, f, {even,odd}, 64*130]
    y2_d = nc.dram_tensor("y2", [CH, 128, 2, TR * COLS], bf16,
                          kind="ExternalOutput").ap()

    with tile.TileContext(nc) as tc:
        with tc.tile_pool(name="pers", bufs=1) as pers, \
             tc.tile_pool(name="wtmp", bufs=3) as wtmp, \
             tc.tile_pool(name="vpool", bufs=3) as vpool, \
             tc.tile_pool(name="drain", bufs=4) as drain, \
             tc.tile_pool(name="psum", bufs=8, space="PSUM") as psum_pool:

            eps_t = pers.tile([128, 1], f32, tag="eps", name="eps_t")
            nc.vector.memset(eps_t[:], 1e-8)
            ones_b = pers.tile([128, 1], bf16, tag="onesb", name="ones_b")
            nc.vector.memset(ones_b[:], 1.0)

            # ---- HAM pre-warm: dummy matmuls during the head DMA wait so
            # the PE clock-gate opens (1.2 -> 2.4 GHz) before the first
            # real matmul; sized to end roughly when weights/V are ready
            dum = pers.tile([128, 256], bf16, tag="dum", name="dum")
            nc.vector.memset(dum[:], 0.0)
            dum_ps = psum_pool.tile([128, 256], f32, tag="pt", name="dum_ps")
            for _ in range(48):
                nc.tensor.matmul(dum_ps[:], dum[:, 0:128], dum[:],
                                 start=True, stop=True)

            # ---- weight + style DMA (sync), x chunks 0-1 on gpsimd ----
            s_t = pers.tile([128, CH], f32, tag="s", name="s_t")
            nc.sync.dma_start(s_t[:], st_d)
            wraw = [pers.tile([128, 9, F], bf16, tag=f"wraw{ch}",
                              name=f"wraw{ch}")
                    for ch in range(CH)]
            nc.sync.dma_start(wraw[0][:], wk_d[0])
            nc.sync.dma_start(wraw[1][:], wk_d[1])

            xt = [pers.tile([128, ROWS, COLS], bf16, tag=f"xt{ch}",
                            name=f"xt{ch}")
                  for ch in range(CH)]
            # first chunk = only the 8 rows V-chunk-0 needs, split across
            # the two idle trigger queues (gpsimd/vector) for fast arrival
            xbounds = [0, 8, 14, 26, 38, 52, 78, 104, 130]
            for bi in range(len(xbounds) - 1):
                ra, rb = xbounds[bi], xbounds[bi + 1]
                for ch in range(CH):
                    q = nc.gpsimd if bi < 2 else nc.sync
                    q.dma_start(
                        xt[ch][:, ra:rb, :],
                        xt_d[ch][:, ra * COLS:rb * COLS]
                        .rearrange("p (a b) -> p a b", a=rb - ra, b=COLS))

            # ---- V planes: vmap[(c0,k,ch)] = (tile, row offset) ----
            vmap = {}

            def emit_v(c0_list, korder=(0, 1, 2, 3)):
                nrows = len(c0_list) * CTR
                r0 = CTR * c0_list[0]
                nr = min(TR, r0 + nrows) - r0
                for k in korder:
                    ia, ib_, op = V_DEFS[k]
                    for ch in range(CH):
                        da = xt[ch][:, 2 * r0 + ia: 2 * r0 + ia + 2 * nr - 1:2, :]
                        db = xt[ch][:, 2 * r0 + ib_: 2 * r0 + ib_ + 2 * nr - 1:2, :]
                        v = vpool.tile([128, VG, COLS], bf16,
                                       tag=f"v{k}_{ch}", name=f"v{k}_{ch}")
                        vv = v[:, :nr, :]
                        if op == "add":
                            nc.vector.tensor_add(vv, da, db)
                        else:
                            nc.vector.tensor_sub(vv, da, db)
                        for j, c0 in enumerate(c0_list):
                            vmap[(c0, k, ch)] = (v, CTR * j)

            # ---- transformed modulated weights (DVE bf16, k-major) ----
            wt = [[[pers.tile([128, F], bf16, tag=f"wt{k}_{kx}_{ch}",
                              name=f"wt{k}_{kx}_{ch}")
                    for ch in range(CH)] for kx in range(3)]
                  for k in range(4)]
            sh_t = pers.tile([128, CH], f32, tag="sh", name="sh_t")
            tsum = [[None] * CH for _ in range(3)]

            def emit_combos(k):
                for kx in range(3):
                    for ch in range(CH):
                        g0 = wraw[ch][:, 3 * kx + 0]
                        g1 = wraw[ch][:, 3 * kx + 1]
                        g2 = wraw[ch][:, 3 * kx + 2]
                        sc = s_t[:, ch:ch + 1]
                        shc = sh_t[:, ch:ch + 1]
                        if k == 0:
                            nc.vector.tensor_scalar_mul(
                                wt[0][kx][ch][:], g0, sc)
                        elif k == 3:
                            nc.vector.tensor_scalar_mul(
                                wt[3][kx][ch][:], g2, sc)
                        elif k == 1:
                            t = wtmp.tile([128, F], bf16, tag=f"t{kx}_{ch}",
                                          name=f"t{kx}_{ch}", bufs=1)
                            nc.vector.tensor_add(t[:], g0, g2)
                            tsum[kx][ch] = t
                            t2 = wtmp.tile([128, F], bf16, tag="t2",
                                           name="t2")
                            nc.vector.tensor_add(t2[:], t[:], g1)
                            nc.vector.tensor_scalar_mul(
                                wt[1][kx][ch][:], t2[:], shc)
                        else:  # k == 2
                            t3 = wtmp.tile([128, F], bf16, tag="t3",
                                           name="t3")
                            nc.vector.tensor_sub(t3[:], tsum[kx][ch][:], g1)
                            nc.vector.tensor_scalar_mul(
                                wt[2][kx][ch][:], t3[:], shc)

            # ---- demod: sq = (wraw*s)^2, one ACT op per c-half ----
            sqall = [pers.tile([128, 9, F], bf16, tag=f"sqall{ch}",
                               name=f"sqall{ch}")
                     for ch in range(CH)]

            def emit_squares(ch):
                nc.scalar.activation(
                    sqall[ch][:].rearrange("p a b -> p (a b)"),
                    wraw[ch][:].rearrange("p a b -> p (a b)"),
                    AF.Square, scale=s_t[:, ch:ch + 1])

            # d2w [128, 2]: column fh; filled by 4 interleaved 9-MM parts.
            # Lives in the main PSUM ring (read once at ~19us; the ring's
            # WAR tracking delays that slot's reuse until after the sqrt).
            d2w = psum_pool.tile([128, 2], f32, tag="pt", name="d2w")

            def emit_d2_part(fh, ch, first):
                for t in range(9):
                    nc.tensor.matmul(
                        d2w[:, fh:fh + 1],
                        sqall[ch][:, t, fh * 128:(fh + 1) * 128],
                        ones_b[:],
                        start=(first and t == 0), stop=(t == 8),
                        skip_group_check=True)

            ivw = pers.tile([128, CH], f32, tag="ivw", name="ivw")
            nivw = pers.tile([128, CH], f32, tag="nivw", name="nivw")

            def emit_invd_finish():
                dm = pers.tile([128, CH], f32, tag="dm", name="dm")
                nc.scalar.activation(dm[:], d2w[:], AF.Sqrt, bias=eps_t[:])
                nc.vector.reciprocal(ivw[:], dm[:])
                nc.vector.tensor_scalar_mul(nivw[:], ivw[:], -1.0)

            # ---- head emission: k0 weights first so MM #1 issues as soon
            # as its (single) weight tile and V0 are both resident ----
            nc.vector.tensor_scalar_add(s_t[:], s_t[:], 1.0)
            nc.vector.tensor_scalar_mul(sh_t[:], s_t[:], 0.5)
            emit_combos(0)
            emit_v([0], korder=(0,))
            emit_combos(3)
            emit_v([0], korder=(3,))
            emit_squares(0)
            emit_combos(1)
            emit_v([0], korder=(1,))
            emit_squares(1)
            emit_combos(2)
            emit_v([0], korder=(2,))
            emit_v([1])

            # ---- main loop: conv groups + fused inverse-transform drain ----
            for c0 in range(NCHUNK):
                r0, r1 = _chunk_rows(c0)
                nr = r1 - r0
                npx = nr * COLS           # 390 (or 130 for the last chunk)
                p0 = r0 * COLS
                for fh in range(CH):
                    first_group = (c0 == 0 and fh == 0)
                    pts = [None] * 4
                    for ki, k in enumerate((0, 3, 1, 2)):
                        pt = psum_pool.tile([128, CTR * COLS + 2], f32,
                                            tag="pt", name=f"pt{k}")
                        # ch-major: a bank's first 3 MMs need only the
                        # c-half-0 weight/V tiles, which land ~2us before
                        # c-half-1's during the ramp
                        for ch in range(CH):
                            for kx in range(3):
                                v, off = vmap[(c0, k, ch)]
                                nc.tensor.matmul(
                                    pt[:, 2 - kx:2 - kx + npx],
                                    wt[k][kx][ch][:, fh * 128:(fh + 1) * 128],
                                    v[:, off:off + nr, :]
                                    .rearrange("p a b -> p (a b)"),
                                    start=(ch == 0 and kx == 0),
                                    stop=(ch == CH - 1 and kx == 2))
                        pts[k] = pt
                        if first_group:
                            # demod tap-sums, interleaved between banks
                            emit_d2_part(ki // 2, ki % 2, ki == 0)
                            # pad the ramp's dependency holes (next bank's
                            # weights/V still in flight) with dummy MMs so
                            # the HAM clock-gate never re-throttles
                            if ki < 3:
                                for _ in range(10):
                                    nc.tensor.matmul(dum_ps[:],
                                                     dum[:, 0:128], dum[:],
                                                     start=True, stop=True)
                    if first_group:
                        emit_invd_finish()
                    # drain: y_e = (M0+M1+M2)*iv, y_o = (M1-M2-M3)*iv
                    nv = npx - 2          # valid cols [2, npx)
                    ivc = ivw[:, fh:fh + 1]
                    s1 = drain.tile([128, CTR * COLS - 2], bf16, tag="s1",
                                    name="s1")
                    nc.scalar.activation(s1[:, :nv], pts[1][:, 2:2 + nv],
                                         AF.Copy, scale=ivc)
                    s2c = drain.tile([128, CTR * COLS - 2], bf16, tag="s2c",
                                     name="s2c")
                    nc.scalar.activation(s2c[:, :nv], pts[2][:, 2:2 + nv],
                                         AF.Copy, scale=ivc)
                    at = drain.tile([128, CTR * COLS - 2], bf16, tag="at",
                                    name="at")
                    nc.vector.scalar_tensor_tensor(
                        at[:, :nv], pts[0][:, 2:2 + nv], ivc,
                        s1[:, :nv], op0=ALU.mult, op1=ALU.add)
                    bt = drain.tile([128, CTR * COLS - 2], bf16, tag="bt",
                                    name="bt")
                    nc.vector.scalar_tensor_tensor(
                        bt[:, :nv], pts[3][:, 2:2 + nv], nivw[:, fh:fh + 1],
                        s1[:, :nv], op0=ALU.mult, op1=ALU.add)
                    y2t = drain.tile([128, 2, CTR * COLS - 2], bf16,
                                     tag="y2t", name="y2t")
                    # final groups: keep the adds on DVE (bf16 2x) so the
                    # tail doesn't serialize through the slower gpsimd hop
                    ae = nc.vector if c0 >= NCHUNK - 2 else nc.gpsimd
                    ae.tensor_add(y2t[:, 0, :nv], at[:, :nv],
                                  s2c[:, :nv])
                    ae.tensor_sub(y2t[:, 1, :nv], bt[:, :nv],
                                  s2c[:, :nv])
                    if c0 >= NCHUNK - 2:
                        # final stores entirely on sync so gpsimd goes
                        # idle early and its pipeline drain overlaps
                        nc.sync.dma_start(
                            y2_d[fh][:, :, p0 + 1:p0 + 1 + nv],
                            y2t[:, :, :nv])
                    else:
                        # steady stores all on gpsimd: the sync queue
                        # carries the bulk x loads, and a store queued
                        # behind a large x transfer stalls the drain ring
                        nc.gpsimd.dma_start(
                            y2_d[fh][:, :, p0 + 1:p0 + 1 + nv],
                            y2t[:, :, :nv])
                # keep V production ~2 chunks ahead of the conv
                if c0 == 0 and NCHUNK > 3:
                    emit_v([2, 3])
                elif c0 % 2 == 1 and c0 + 3 < NCHUNK:
                    emit_v([c0 + 3, c0 + 4] if c0 + 4 < NCHUNK
                           else [c0 + 3])

    nc.compile()
    return nc


def _get_nc():
    if "nc" not in _COMPILED:
        _COMPILED["nc"] = _build_nc()
    return _COMPILED["nc"]


def _prep_in_maps(x, style, kernel):
    """Host-side layout marshalling: shard over B, transpose+pad x."""
    import ml_dtypes
    x = np.ascontiguousarray(x, dtype=np.float32)
    style = np.ascontiguousarray(style, dtype=np.float32)
    kernel = np.ascontiguousarray(kernel, dtype=np.float32)
    # [3(ky),3(kx),C,F] -> [ch, c, t=3*kx+ky, f], bf16
    wk = np.ascontiguousarray(
        kernel.reshape(3, 3, CH, 128, F).transpose(2, 3, 1, 0, 4)
        .reshape(CH, 128, 9, F).astype(ml_dtypes.bfloat16))
    in_maps = []
    for b in range(B):
        xp = np.zeros((C, ROWS, COLS), dtype=ml_dtypes.bfloat16)
        xp[:, 1:H + 1, 1:W + 1] = x[b].transpose(2, 0, 1)
        xt = np.ascontiguousarray(xp.reshape(CH, 128, ROWS * COLS))
        st = np.ascontiguousarray(style[b].reshape(CH, 128).T)
        in_maps.append({"xt": xt, "st": st, "wk": wk})
    return in_maps


def run_cores(x, style, kernel, trace=False, trace_cores=None):
    """Compile (cached) + run on the 8 NeuronCores. Returns (y, results)."""
    from concourse.bass_utils import run_bass_kernel_spmd

    nc = _get_nc()
    in_maps = _prep_in_maps(x, style, kernel)
    kwargs = {}
    if trace:
        kwargs.update(trace=True, trace_cores=trace_cores)
    res = run_bass_kernel_spmd(nc, in_maps, list(range(N_CORES)), **kwargs)
    # y2 [fh, 128, {even,odd}, 64*130]: rows 2i / 2i+1, pad cols stripped
    y = np.empty((B, H, W, F), dtype=np.float32)
    for b in range(B):
        pl = res.results[b]["y2"].reshape(CH * 128, 2, TR, COLS)\
            .astype(np.float32)
        y[b, 0::2] = pl[:, 0, :, 1:W + 1].transpose(1, 2, 0)
        y[b, 1::2] = pl[:, 1, :, 1:W + 1].transpose(1, 2, 0)
    return y, res


def kernel(x, style, kernel):
    y, _ = run_cores(x, style, kernel)
    return y.astype(np.float32)


# revision 36
# speedup vs baseline: 1.0069x; 1.0069x over previous
"""StyleGAN-style modulated 3x3 conv on 8 Trainium2 NeuronCores.

Problem: y = conv2d(x, kernel * (style+1) / demod), SAME padding,
  x [B=8, H=128, W=128, C=256], kernel [3,3,C=256,F=256],
  style [B,1,1,C], demod[f] = sqrt(sum_{ky,kx,c} wmod^2 + 1e-8).

Sharding: data-parallel over batch B - each of the 8 cores convolves one
sample with its own modulated/demodulated kernel. No cross-core comm.

Algorithm: 1-D Winograd F(2,3) along H. The 3 ky taps collapse into 4
transformed planes, so each output row-pair needs 4x3(kx)x2(ch) = 24
matmul passes instead of the direct conv's 36: PE streamed columns drop
1.5x (599k -> 399k per core, ~166us at 2.4 GHz).

  forward (DVE, bf16 2x): per tile-row i (output rows 2i, 2i+1), with
    d_a = xp[2i+a] (xp = zero-padded [C,130,130] bf16 image):
      V0 = d0-d2, V1 = d1+d2, V2 = d2-d1, V3 = d1-d3
    produced per-chunk for the first two chunks (fast availability at
    the head), then two chunks per op.
  matmuls (PE): M_k[f,p] += sum_{kx,c} wt[k,kx,c,f] * V_k[c,p+kx-1],
    kx shift applied on the PSUM *output* AP (out cols 2-kx..) so the
    moving operand is one contiguous [128,390] AP per (k,ch) and tile
    seams land on junk pad columns. Banks filled in order k=0,3,1,2 to
    match weight-combo readiness at the head.
  weights (DVE bf16, head, k-ordered): wt0 = g0*s, wt3 = g2*s,
    t = g0+g2, t2 = t+g1, t3 = t-g1, wt1 = t2*s/2, wt2 = t3*s/2.
  demod: sq = ACT.Square(wraw, scale=s) one op per c-half (= wmod^2,
    bf16); d2[f, fh] accumulated into one [128,2] PSUM tile via 36 tiny
    matmuls against a ones column, emitted in 4 parts interleaved with
    the first group's banks so invd is ready before the first drain;
    invd = 1/sqrt(d2+1e-8).
  inverse (ACT+DVE+GPSIMD, fused into PSUM drain, demod applied as
  per-partition scale):
    y_even = (M0+M1+M2)*invd, y_odd = (M1-M2-M3)*invd via
      s1 = ACT.copy(M1, scale=iv), s2 = ACT.copy(M2, scale=iv)
      a = DVE.STT(M0*iv + s1);  b = DVE.STT(M3*(-iv) + s1)
      y_e = GPSIMD.add(a, s2);  y_o = GPSIMD.sub(b, s2)
    y_e/y_o written to halves of one tile -> single DMA per group,
    alternating gpsimd/sync trigger queues.

Host does layout-only marshalling: shard over B, transpose+zero-pad x
to [ch,128,130,130] bf16, reorder kernel taps kx-major (bf16); gather
interleaves even/odd output row planes and strips pad columns.

Measured: ~197.8-198.7 us HW exec (8 cores, traced), rel err 5.4e-3, vs
the direct-conv baseline's 285.6 us / 2.3e-3 (same measurement): 1.44x.
Breakdown: ~7 us fixed NEFF preamble, ~9 us head (DMA latency + V/
weight prep, PE HAM-prewarmed by dummy matmuls), ~177 us PE-bound MM
region (>99% tensor-engine occupancy, streaming floor 166 us), ~14 us
drain tail + NEFF epilogue. Occasional ~20% slower runs are the chip's
P0 power-state downclock (2.4 -> 2.0 GHz), not kernel-dependent.
"""

import sys
import os

for _p in ("/opt/trn_rl_repo", "/root/.axon_site", "/root/.axon_site/_ro/trn_rl_repo",
           "/root/.axon_site/_ro/pypackages"):
    if os.path.isdir(_p) and _p not in sys.path:
        sys.path.append(_p)

import numpy as np

B, H, W, C, F = 8, 128, 128, 256, 256
CH = C // 128                  # c-half count (contraction tiled by 128)
ROWS, COLS = H + 2, W + 2      # padded image dims
TR = H // 2                    # 64 Winograd tile-rows (2 output rows each)
CTR = 3                        # tile-rows per PSUM group (3*130=390 <= 512)
NCHUNK = (TR + CTR - 1) // CTR # 22 (21 full + 1 single-row)
VG = 2 * CTR                   # tile-rows per steady-state V op (2 chunks)
N_CORES = 8

_COMPILED = {}

V_DEFS = [(0, 2, "sub"), (1, 2, "add"), (2, 1, "sub"), (1, 3, "sub")]


def _chunk_rows(c0):
    r0 = CTR * c0
    r1 = min(TR, r0 + CTR)
    return r0, r1


def _build_nc():
    import concourse.bacc as bacc
    import concourse.mybir as mybir
    import concourse.tile as tile

    f32 = mybir.dt.float32
    bf16 = mybir.dt.bfloat16
    AF = mybir.ActivationFunctionType
    ALU = mybir.AluOpType

    nc = bacc.Bacc("TRN2", target_bir_lowering=False, debug=False,
                   num_devices=N_CORES)

    xt_d = nc.dram_tensor("xt", [CH, 128, ROWS * COLS], bf16,
                          kind="ExternalInput").ap()
    st_d = nc.dram_tensor("st", [128, CH], f32, kind="ExternalInput").ap()
    # raw weights bf16, tap axis kx-major (t = 3*kx + ky)
    wk_d = nc.dram_tensor("wk", [CH, 128, 9, F], bf16,
                          kind="ExternalInput").ap()
    # merged even/odd output row planes [f_half, f, {even,odd}, 64*130]
    y2_d = nc.dram_tensor("y2", [CH, 128, 2, TR * COLS], bf16,
                          kind="ExternalOutput").ap()

    with tile.TileContext(nc) as tc:
        with tc.tile_pool(name="pers", bufs=1) as pers, \
             tc.tile_pool(name="wtmp", bufs=3) as wtmp, \
             tc.tile_pool(name="vpool", bufs=3) as vpool, \
             tc.tile_pool(name="drain", bufs=4) as drain, \
             tc.tile_pool(name="psum", bufs=8, space="PSUM") as psum_pool:

            eps_t = pers.tile([128, 1], f32, tag="eps", name="eps_t")
            nc.vector.memset(eps_t[:], 1e-8)
            ones_b = pers.tile([128, 1], bf16, tag="onesb", name="ones_b")
            nc.vector.memset(ones_b[:], 1.0)

            # ---- HAM pre-warm: dummy matmuls during the head DMA wait so
            # the PE clock-gate opens (1.2 -> 2.4 GHz) before the first
            # real matmul; sized to end roughly when weights/V are ready
            dum = pers.tile([128, 256], bf16, tag="dum", name="dum")
            nc.vector.memset(dum[:], 0.0)
            dum_ps = psum_pool.tile([128, 256], f32, tag="pt", name="dum_ps")
            for _ in range(48):
                nc.tensor.matmul(dum_ps[:], dum[:, 0:128], dum[:],
                                 start=True, stop=True)

            # ---- weight + style DMA (sync), x chunks 0-1 on gpsimd ----
            s_t = pers.tile([128, CH], f32, tag="s", name="s_t")
            nc.sync.dma_start(s_t[:], st_d)
            wraw = [pers.tile([128, 9, F], bf16, tag=f"wraw{ch}",
                              name=f"wraw{ch}")
                    for ch in range(CH)]
            nc.sync.dma_start(wraw[0][:], wk_d[0])
            nc.sync.dma_start(wraw[1][:], wk_d[1])

            xt = [pers.tile([128, ROWS, COLS], bf16, tag=f"xt{ch}",
                            name=f"xt{ch}")
                  for ch in range(CH)]
            # first chunk = only the 8 rows V-chunk-0 needs, split across
            # the two idle trigger queues (gpsimd/vector) for fast arrival
            xbounds = [0, 8, 14, 26, 38, 52, 78, 104, 130]
            for bi in range(len(xbounds) - 1):
                ra, rb = xbounds[bi], xbounds[bi + 1]
                for ch in range(CH):
                    q = nc.gpsimd if bi < 2 else nc.sync
                    q.dma_start(
                        xt[ch][:, ra:rb, :],
                        xt_d[ch][:, ra * COLS:rb * COLS]
                        .rearrange("p (a b) -> p a b", a=rb - ra, b=COLS))

            # ---- V planes: vmap[(c0,k,ch)] = (tile, row offset) ----
            vmap = {}

            def emit_v(c0_list, korder=(0, 1, 2, 3)):
                nrows = len(c0_list) * CTR
                r0 = CTR * c0_list[0]
                nr = min(TR, r0 + nrows) - r0
                for k in korder:
                    ia, ib_, op = V_DEFS[k]
                    for ch in range(CH):
                        da = xt[ch][:, 2 * r0 + ia: 2 * r0 + ia + 2 * nr - 1:2, :]
                        db = xt[ch][:, 2 * r0 + ib_: 2 * r0 + ib_ + 2 * nr - 1:2, :]
                        v = vpool.tile([128, VG, COLS], bf16,
                                       tag=f"v{k}_{ch}", name=f"v{k}_{ch}")
                        vv = v[:, :nr, :]
                        if op == "add":
                            nc.vector.tensor_add(vv, da, db)
                        else:
                            nc.vector.tensor_sub(vv, da, db)
                        for j, c0 in enumerate(c0_list):
                            vmap[(c0, k, ch)] = (v, CTR * j)

            # ---- transformed modulated weights (DVE bf16, k-major) ----
            wt = [[[pers.tile([128, F], bf16, tag=f"wt{k}_{kx}_{ch}",
                              name=f"wt{k}_{kx}_{ch}")
                    for ch in range(CH)] for kx in range(3)]
                  for k in range(4)]
            sh_t = pers.tile([128, CH], f32, tag="sh", name="sh_t")
            tsum = [[None] * CH for _ in range(3)]

            def emit_combos(k):
                for kx in range(3):
                    for ch in range(CH):
                        g0 = wraw[ch][:, 3 * kx + 0]
                        g1 = wraw[ch][:, 3 * kx + 1]
                        g2 = wraw[ch][:, 3 * kx + 2]
                        sc = s_t[:, ch:ch + 1]
                        shc = sh_t[:, ch:ch + 1]
                        if k == 0:
                            nc.vector.tensor_scalar_mul(
                                wt[0][kx][ch][:], g0, sc)
                        elif k == 3:
                            nc.vector.tensor_scalar_mul(
                                wt[3][kx][ch][:], g2, sc)
                        elif k == 1:
                            t = wtmp.tile([128, F], bf16, tag=f"t{kx}_{ch}",
                                          name=f"t{kx}_{ch}", bufs=1)
                            nc.vector.tensor_add(t[:], g0, g2)
                            tsum[kx][ch] = t
                            t2 = wtmp.tile([128, F], bf16, tag="t2",
                                           name="t2")
                            nc.vector.tensor_add(t2[:], t[:], g1)
                            nc.vector.tensor_scalar_mul(
                                wt[1][kx][ch][:], t2[:], shc)
                        else:  # k == 2
                            t3 = wtmp.tile([128, F], bf16, tag="t3",
                                           name="t3")
                            nc.vector.tensor_sub(t3[:], tsum[kx][ch][:], g1)
                            nc.vector.tensor_scalar_mul(
                                wt[2][kx][ch][:], t3[:], shc)

            # ---- demod: sq = (wraw*s)^2, one ACT op per c-half ----
            sqall = [pers.tile([128, 9, F], bf16, tag=f"sqall{ch}",
                               name=f"sqall{ch}")
                     for ch in range(CH)]

            def emit_squares(ch):
                nc.scalar.activation(
                    sqall[ch][:].rearrange("p a b -> p (a b)"),
                    wraw[ch][:].rearrange("p a b -> p (a b)"),
                    AF.Square, scale=s_t[:, ch:ch + 1])

            # d2w [128, 2]: column fh; filled by 4 interleaved 9-MM parts.
            # Lives in the main PSUM ring (read once at ~19us; the ring's
            # WAR tracking delays that slot's reuse until after the sqrt).
            d2w = psum_pool.tile([128, 2], f32, tag="pt", name="d2w")

            def emit_d2_part(fh, ch, first):
                for t in range(9):
                    nc.tensor.matmul(
                        d2w[:, fh:fh + 1],
                        sqall[ch][:, t, fh * 128:(fh + 1) * 128],
                        ones_b[:],
                        start=(first and t == 0), stop=(t == 8),
                        skip_group_check=True)

            ivw = pers.tile([128, CH], f32, tag="ivw", name="ivw")
            nivw = pers.tile([128, CH], f32, tag="nivw", name="nivw")

            def emit_invd_finish():
                dm = pers.tile([128, CH], f32, tag="dm", name="dm")
                nc.scalar.activation(dm[:], d2w[:], AF.Sqrt, bias=eps_t[:])
                nc.vector.reciprocal(ivw[:], dm[:])
                nc.vector.tensor_scalar_mul(nivw[:], ivw[:], -1.0)

            # ---- head emission: k0 weights first so MM #1 issues as soon
            # as its (single) weight tile and V0 are both resident ----
            nc.vector.tensor_scalar_add(s_t[:], s_t[:], 1.0)
            nc.vector.tensor_scalar_mul(sh_t[:], s_t[:], 0.5)
            emit_combos(0)
            emit_v([0], korder=(0,))
            emit_combos(3)
            emit_v([0], korder=(3,))
            emit_squares(0)
            emit_combos(1)
            emit_v([0], korder=(1,))
            emit_squares(1)
            emit_combos(2)
            emit_v([0], korder=(2,))
            emit_v([1])

            # ---- main loop: conv groups + fused inverse-transform drain ----
            for c0 in range(NCHUNK):
                r0, r1 = _chunk_rows(c0)
                nr = r1 - r0
                npx = nr * COLS           # 390 (or 130 for the last chunk)
                p0 = r0 * COLS
                for fh in range(CH):
                    first_group = (c0 == 0 and fh == 0)
                    pts = [None] * 4
                    for ki, k in enumerate((0, 3, 1, 2)):
                        pt = psum_pool.tile([128, CTR * COLS + 2], f32,
                                            tag="pt", name=f"pt{k}")
                        # ch-major: a bank's first 3 MMs need only the
                        # c-half-0 weight/V tiles, which land ~2us before
                        # c-half-1's during the ramp
                        for ch in range(CH):
                            for kx in range(3):
                                v, off = vmap[(c0, k, ch)]
                                nc.tensor.matmul(
                                    pt[:, 2 - kx:2 - kx + npx],
                                    wt[k][kx][ch][:, fh * 128:(fh + 1) * 128],
                                    v[:, off:off + nr, :]
                                    .rearrange("p a b -> p (a b)"),
                                    start=(ch == 0 and kx == 0),
                                    stop=(ch == CH - 1 and kx == 2))
                        pts[k] = pt
                        if first_group:
                            # demod tap-sums, interleaved between banks
                            emit_d2_part(ki // 2, ki % 2, ki == 0)
                            # pad the ramp's dependency holes (next bank's
                            # weights/V still in flight) with dummy MMs so
                            # the HAM clock-gate never re-throttles
                            if ki < 2:
                                for _ in range(8):
                                    nc.tensor.matmul(dum_ps[:],
                                                     dum[:, 0:128], dum[:],
                                                     start=True, stop=True)
                    if first_group:
                        emit_invd_finish()
                    # drain: y_e = (M0+M1+M2)*iv, y_o = (M1-M2-M3)*iv
                    nv = npx - 2          # valid cols [2, npx)
                    ivc = ivw[:, fh:fh + 1]
                    s1 = drain.tile([128, CTR * COLS - 2], bf16, tag="s1",
                                    name="s1")
                    nc.scalar.activation(s1[:, :nv], pts[1][:, 2:2 + nv],
                                         AF.Copy, scale=ivc)
                    s2c = drain.tile([128, CTR * COLS - 2], bf16, tag="s2c",
                                     name="s2c")
                    nc.scalar.activation(s2c[:, :nv], pts[2][:, 2:2 + nv],
                                         AF.Copy, scale=ivc)
                    at = drain.tile([128, CTR * COLS - 2], bf16, tag="at",
                                    name="at")
                    nc.vector.scalar_tensor_tensor(
                        at[:, :nv], pts[0][:, 2:2 + nv], ivc,
                        s1[:, :nv], op0=ALU.mult, op1=ALU.add)
                    bt = drain.tile([128, CTR * COLS - 2], bf16, tag="bt",
                                    name="bt")
                    nc.vector.scalar_tensor_tensor(
                        bt[:, :nv], pts[3][:, 2:2 + nv], nivw[:, fh:fh + 1],
                        s1[:, :nv], op0=ALU.mult, op1=ALU.add)
                    y2t = drain.tile([128, 2, CTR * COLS - 2], bf16,
                                     tag="y2t", name="y2t")
                    # final groups: keep the adds on DVE (bf16 2x) so the
                    # tail doesn't serialize through the slower gpsimd hop
                    ae = nc.vector if c0 >= NCHUNK - 2 else nc.gpsimd
                    ae.tensor_add(y2t[:, 0, :nv], at[:, :nv],
                                  s2c[:, :nv])
                    ae.tensor_sub(y2t[:, 1, :nv], bt[:, :nv],
                                  s2c[:, :nv])
                    if c0 >= NCHUNK - 2:
                        # final stores entirely on sync so gpsimd goes
                        # idle early and its pipeline drain overlaps
                        nc.sync.dma_start(
                            y2_d[fh][:, :, p0 + 1:p0 + 1 + nv],
                            y2t[:, :, :nv])
                    else:
                        # steady stores all on gpsimd: the sync queue
                        # carries the bulk x loads, and a store queued
                        # behind a large x transfer stalls the drain ring
                        nc.gpsimd.dma_start(
                            y2_d[fh][:, :, p0 + 1:p0 + 1 + nv],
                            y2t[:, :, :nv])
                # keep V production ~2 chunks ahead of the conv
                if c0 == 0 and NCHUNK > 3:
                    emit_v([2, 3])
                elif c0 % 2 == 1 and c0 + 3 < NCHUNK:
                    emit_v([c0 + 3, c0 + 4] if c0 + 4 < NCHUNK
                           else [c0 + 3])

    nc.compile()
    return nc


def _get_nc():
    if "nc" not in _COMPILED:
        _COMPILED["nc"] = _build_nc()
    return _COMPILED["nc"]


def _prep_in_maps(x, style, kernel):
    """Host-side layout marshalling: shard over B, transpose+pad x."""
    import ml_dtypes
    x = np.ascontiguousarray(x, dtype=np.float32)
    style = np.ascontiguousarray(style, dtype=np.float32)
    kernel = np.ascontiguousarray(kernel, dtype=np.float32)
    # [3(ky),3(kx),C,F] -> [ch, c, t=3*kx+ky, f], bf16
    wk = np.ascontiguousarray(
        kernel.reshape(3, 3, CH, 128, F).transpose(2, 3, 1, 0, 4)
        .reshape(CH, 128, 9, F).astype(ml_dtypes.bfloat16))
    in_maps = []
    for b in range(B):
        xp = np.zeros((C, ROWS, COLS), dtype=ml_dtypes.bfloat16)
        xp[:, 1:H + 1, 1:W + 1] = x[b].transpose(2, 0, 1)
        xt = np.ascontiguousarray(xp.reshape(CH, 128, ROWS * COLS))
        st = np.ascontiguousarray(style[b].reshape(CH, 128).T)
        in_maps.append({"xt": xt, "st": st, "wk": wk})
    return in_maps


def run_cores(x, style, kernel, trace=False, trace_cores=None):
    """Compile (cached) + run on the 8 NeuronCores. Returns (y, results)."""
    from concourse.bass_utils import run_bass_kernel_spmd

    nc = _get_nc()
    in_maps = _prep_in_maps(x, style, kernel)
    kwargs = {}
    if trace:
        kwargs.update(trace=True, trace_cores=trace_cores)
    res = run_bass_kernel_spmd(nc, in_maps, list(range(N_CORES)), **kwargs)
    # y2 [fh, 128, {even,odd}, 64*130]: rows 2i / 2i+1, pad cols stripped
    y = np.empty((B, H, W, F), dtype=np.float32)
    for b in range(B):
        pl = res.results[b]["y2"].reshape(CH * 128, 2, TR, COLS)\
            .astype(np.float32)
        y[b, 0::2] = pl[:, 0, :, 1:W + 1].transpose(1, 2, 0)
        y[b, 1::2] = pl[:, 1, :, 1:W + 1].transpose(1, 2, 0)
    return y, res


def kernel(x, style, kernel):
    y, _ = run_cores(x, style, kernel)
    return y.astype(np.float32)
